# revision 26
# baseline (speedup 1.0000x reference)
"""Cross-modal contrastive loss on 8 Trainium2 NeuronCores.

Strategy (B=8192, d=256 hardcoded):
  * Host sorts rows by patient id (same-patient mask becomes a narrow band),
    scales projections by 8 and quantizes to fp8-e4m3.
  * Each core owns a 1024-row slice of z_a and the full z_t. Main matmuls run
    in fp8 DoubleRow mode (full 256-contraction in one pass, 2x PE rate).
  * exp(sim/T - C) with a fixed offset C (|sim|<=1) is computed per
    (128, 2048) tile either on ACT (table exp -> bf16) or on DVE via the
    Schraudolph bit trick (i16 = A*psum + B, reinterpreted as bf16).
  * Column sums: per block, a serial DVE fold chain accumulates the 8 row
    tiles into a bf16 colacc; a ones-matmul partition-reduces it to [1,2048]
    psum which is DMA'd straight to HBM.
  * Row sums: every fold carries accum_out (sum over the free dim of the
    fold OUTPUT = cumulative block-row-sums); the host recovers per-tile row
    sums by differencing consecutive fold accums. Row-tile 0 of each block
    uses the ACT activation's accum_out directly.
  * The same-patient band is recomputed over a small window with identical
    fp8/ACT arithmetic and subtracted on the host. Per-core column-block
    permutation puts the band columns always in local blocks 0/1, so one
    SPMD program serves all cores (Schraudolph tiles live in blocks 2/3,
    never overlapping band columns).
"""

import numpy as np
import ml_dtypes

TEMPERATURE = 0.03
SCALE = 1.0 / TEMPERATURE
C = SCALE + 0.01  # fixed logsumexp offset; logits are <= SCALE * (1 + eps)
B = 8192
D = 256
NCORES = 8
ROWS = B // NCORES          # 1024 rows per core
RT = ROWS // 128            # 8 row-tiles per core
NB = 4                      # column blocks of 2048
CPB = 4                     # 512-chunks per block
PAD, W = 64, 256
Wb = ROWS + 2 * PAD
BF16 = ml_dtypes.bfloat16
FP8 = ml_dtypes.float8_e4m3

PSC = SCALE / 64.0          # psum holds 64*sim (inputs pre-scaled by 8)
# Schraudolph constants: i16 = trunc(S1 * psum + S2) viewed as bf16
# approximates exp(PSC*psum - C).  S1 = (128/ln2)*PSC,
# S2 = 16256 + 0.5(trunc->round) - 128*sigma - (128/ln2)*C
SCH_A = 128.0 / np.log(2.0)
SCH_SIGMA = 0.0575
S1 = SCH_A * PSC
S2 = 16256.5 - 128.0 * SCH_SIGMA - SCH_A * C
# (block, row-tile) pairs exp'd on DVE via Schraudolph. Band columns live in
# local block 0 (all cores) and (r=7, block 1) (odd cores), which must use
# the ACT path (bitwise match with the band phase); everything else is fair
# game. Spread across the timeline to balance ACT/DVE.
# Per-block Schraudolph chain-prefix length: tiles r=1..k of each block are
# exp'd on DVE; their folds are STT-with-accum whose cumulative row sums are
# differenced on the host. Block 0 holds the band columns -> ACT only.
K_BLK = [0, 3, 3, 3]
SCH_TILES = {(b, r) for b in range(4) for r in range(1, K_BLK[b] + 1)}

_LDW_PATCHED = False


def _install_ldw_opt_patch():
    """walrus ships with --enable-ldw-opt=false; with 8 consecutive
    same-weight matmuls per row-tile the dedupe is a big PE win."""
    global _LDW_PATCHED
    if _LDW_PATCHED:
        return
    import concourse.bass_utils as bu

    orig = bu.run_command

    def patched(cmd, **kw):
        if isinstance(cmd, list):
            cmd = ["--enable-ldw-opt=true" if c == "--enable-ldw-opt=false"
                   else c for c in cmd]
        return orig(cmd, **kw)

    bu.run_command = patched
    _LDW_PATCHED = True

_CACHE = {}


def _install_drain_patch():
    """walrus accepts at most one sync-wait per CTRL instruction, but
    TileContext's exit drain collects one wait per outstanding semaphore.
    Spread the waits across nop instructions, one wait each."""
    import bass_rust
    import concourse.tile as tile_mod
    from concourse.vector_clock import ScopedClock

    if getattr(tile_mod.TileContext, "_drain_patch_installed", False):
        return

    def _patched(self, tick_clock, wait_clock):
        nc = self.nc
        probe = nc.sync.nop(nofuse=True)
        wait_clock.add_sem_waits(
            probe.ins, ScopedClock({None: tick_clock.global_clock})
        )
        si = probe.ins.sync_info
        waits = list(si.on_wait) if si is not None else []
        if len(waits) > 1:
            si.on_wait = waits[:1]
            for w in waits[1:]:
                extra = nc.sync.nop(nofuse=True)
                extra.ins.sync_info = bass_rust.SyncInfo(on_wait=[w], on_update=[])
        nc.sync.drain()
        nc.all_engine_barrier()
        popped = nc._tile_sem_poison_stack.pop()
        assert popped is self._sem_poison
        nc.clear_and_free_semaphores(list(self.sems.allocated().values()))
        nc.all_engine_barrier()

    tile_mod.TileContext._drain_and_barrier = _patched
    tile_mod.TileContext._drain_patch_installed = True


def _split_multi_waits(nc):
    """walrus in this container accepts at most one sync-wait per instruction.
    Hoist extra waits onto same-engine nops inserted just before the
    instruction (engine streams are in-order, so the waits still gate it)."""
    import bass_rust

    n = 0
    for fn in nc.m.functions:
        for bb in fn.blocks:
            insts = list(bb.instructions)
            out = []
            for inst in insts:
                si = inst.sync_info
                if si is not None and len(si.on_wait) > 1:
                    waits = list(si.on_wait)
                    for w in waits[:-1]:
                        n += 1
                        nop = bass_rust.InstNoOp(
                            name=f"I-waitsplit-{n}", ins=[], outs=[]
                        )
                        nop.engine = inst.engine
                        nop.sync_info = bass_rust.SyncInfo(
                            on_wait=[w], on_update=[]
                        )
                        out.append(nop)
                    si.on_wait = waits[-1:]
                out.append(inst)
            if n:
                bb.instructions = out
    return n


def _build_program(split_waits=True):
    from contextlib import ExitStack
    import concourse.bass as bass
    import concourse.tile as tile
    from concourse import mybir

    _install_drain_patch()

    nc = bass.Bass()
    bf = mybir.dt.bfloat16
    f32 = mybir.dt.float32
    fp8 = mybir.dt.float8e4
    i16 = mybir.dt.int16
    DR = mybir.MatmulPerfMode.DoubleRowSwInterleave

    # Drop preamble memsets for const APs this program never uses.
    drop = ("const-float32-1.0", "const-bfloat16-1.0", "const-uint8-127")
    bb0 = nc.m.functions[0].blocks[0]
    bb0.instructions = [
        i for i in bb0.instructions
        if not (i.opcode == "Memset"
                and any(d in str(i.outs[0]) for d in drop))
    ]

    zaT = nc.declare_dram_parameter("zaT", [128, RT, 256], fp8, isOutput=False)
    ztT = nc.declare_dram_parameter("ztT", [NB, 128, 2, CPB, 512], fp8, isOutput=False)
    ztTb = nc.declare_dram_parameter("ztTb", [128, 2, Wb], fp8, isOutput=False)
    maskb = nc.declare_dram_parameter("maskb", [128, RT, W], bf, isOutput=False)

    rowacc = nc.declare_dram_parameter("rowacc", [128, NB * RT], f32, isOutput=True)
    bandrow = nc.declare_dram_parameter("bandrow", [128, RT], f32, isOutput=True)
    colacc = nc.declare_dram_parameter("colacc", [NB, 128, 2048], bf, isOutput=True)
    bandstk = nc.declare_dram_parameter("bandstk", [128, RT * W], bf, isOutput=True)

    with ExitStack() as ctx:
        tc = ctx.enter_context(tile.TileContext(nc))
        singles = ctx.enter_context(tc.tile_pool(name="singles", bufs=1))
        ztpool = ctx.enter_context(tc.tile_pool(name="ztpool", bufs=4))
        exppool = ctx.enter_context(tc.tile_pool(name="exppool", bufs=7))
        spool = ctx.enter_context(tc.tile_pool(name="spool", bufs=3))
        capool = ctx.enter_context(tc.tile_pool(name="capool", bufs=5))
        ppool = ctx.enter_context(tc.tile_pool(name="ppool", bufs=4))
        bpool = ctx.enter_context(tc.tile_pool(name="bpool", bufs=2))

        biasC = singles.tile([128, 1], f32)
        nc.vector.memset(biasC[:], -C)
        # Dummy activation: pull the exp table load off the critical path.
        warm = singles.tile([128, 1], f32)
        nc.scalar.activation(warm[:], biasC[:], mybir.ActivationFunctionType.Exp)

        rowacc_sb = singles.tile([128, NB * RT], f32)
        bandrow_sb = singles.tile([128, RT], f32)
        bandstack = singles.tile([128, RT, W], bf)

        # DMA order: band phase (first compute) needs zaT + the band windows.
        zaT0 = singles.tile([128, 2, 256], fp8)
        nc.sync.dma_start(zaT0[:], zaT[:, 0:2, :])
        ztTbp = []
        for p in range(RT // 2):
            t = singles.tile([128, 2, 384], fp8, name=f"ztTbp{p}")
            nc.sync.dma_start(t[:], ztTb[:, :, 256 * p:256 * p + 384])
            ztTbp.append(t)
            if p == 0:
                zaT1 = singles.tile([128, 6, 256], fp8)
                nc.sync.dma_start(zaT1[:], zaT[:, 2:RT, :])
        maskb_sb = singles.tile([128, RT, W], bf)
        nc.gpsimd.dma_start(maskb_sb[:], maskb[:])

        def za_dr(r):
            # [128, 256] SW-interleaved DoubleRow stationary for row-tile r
            if r < 2:
                return zaT0[:, r, :]
            return zaT1[:, r - 2, :]

        pmain = ctx.enter_context(tc.tile_pool(name="pmain", bufs=2, space="PSUM"))

        # ---- band phase: same-patient window, identical fp8/ACT arithmetic
        for r0 in range(0, RT, 2):
            pb = pmain.tile([128, 2048], f32, tag="pm")
            for i in range(2):
                nc.tensor.matmul(
                    pb[:, i * W:(i + 1) * W],
                    za_dr(r0 + i),
                    ztTbp[r0 // 2][:, :, i * 128:i * 128 + W],
                    start=True, stop=True,
                    perf_mode=DR,
                    skip_group_check=True,
                )
            exp_b = bpool.tile([128, 2 * W], bf, tag="exp_b")
            nc.scalar.activation(
                exp_b[:], pb[:, :2 * W], mybir.ActivationFunctionType.Exp,
                bias=biasC[:], scale=PSC,
            )
            for i in range(2):
                nc.vector.scalar_tensor_tensor(
                    out=bandstack[:, r0 + i, :],
                    in0=exp_b[:, i * W:(i + 1) * W],
                    scalar=1.0,
                    in1=maskb_sb[:, r0 + i, :],
                    op0=mybir.AluOpType.mult,
                    op1=mybir.AluOpType.mult,
                    accum_out=bandrow_sb[:, r0 + i:r0 + i + 1],
                )

        # band column partial sums: ship bandstack to HBM, reduce on host
        nc.sync.dma_start(
            bandstk[:], bandstack[:].rearrange("p r w -> p (r w)")
        )

        # ---- main phase: 4 column blocks of 2048, processed as two pairs
        # (r-major within a pair so the PE reuses each row-tile's weights
        # for 8 consecutive matmuls -> LDWEIGHTS fully hidden).
        ztbs = []
        for b in range(NB):
            ztb = ztpool.tile([128, 2, CPB, 512], fp8, tag="ztb")
            nc.sync.dma_start(ztb[:], ztT[b])
            ztbs.append(ztb)
        E = {}       # (b, r) -> AP of an ACT-exp tile
        ch = {}      # b -> STT chain tile (sch prefix)
        pr = {}      # (b, tag) -> pair tiles
        for half in range(2):
            for r in range(RT):
                for b in (2 * half, 2 * half + 1):
                    ztb = ztbs[b]
                    k = K_BLK[b]
                    pm = pmain.tile([128, 2048], f32, tag="pm")
                    for jj in range(CPB):
                        nc.tensor.matmul(
                            pm[:, jj * 512:(jj + 1) * 512],
                            za_dr(r),
                            ztb[:, :, jj, :],
                            start=True, stop=True,
                            perf_mode=DR,
                            skip_group_check=True,
                        )
                    racc = rowacc_sb[:, b * RT + r:b * RT + r + 1]
                    if (b, r) in SCH_TILES:
                        st = spool.tile([128, 2048], i16, tag="sch")
                        nc.vector.tensor_scalar(
                            st[:], pm[:], S1, S2,
                            mybir.AluOpType.mult, mybir.AluOpType.add,
                        )
                        # fold into the chain; accum = cumulative row sums
                        # (host recovers per-tile values by differencing)
                        ca2 = capool.tile([128, 2048], bf, tag="ca")
                        nc.vector.scalar_tensor_tensor(
                            out=ca2[:],
                            in0=st[:].bitcast(bf),
                            scalar=1.0,
                            in1=E[(b, 0)] if r == 1 else ch[b],
                            op0=mybir.AluOpType.mult,
                            op1=mybir.AluOpType.add,
                            accum_out=racc,
                        )
                        ch[b] = ca2[:]
                    else:
                        ex = exppool.tile([128, 2048], bf, tag="exp")
                        nc.scalar.activation(
                            ex[:], pm[:], mybir.ActivationFunctionType.Exp,
                            bias=biasC[:], scale=PSC,
                            accum_out=racc,
                        )
                        E[(b, r)] = ex[:]
                    # static fold schedule (Pool takes the independent pairs)
                    if k == 0:
                        if r == 1:
                            p = capool.tile([128, 2048], bf, tag="ca")
                            nc.vector.tensor_add(p[:], E[(b, 0)], E[(b, 1)])
                            pr[(b, 0)] = p[:]
                        elif r == 3:
                            p = ppool.tile([128, 2048], bf, tag="pp")
                            nc.gpsimd.tensor_add(p[:], E[(b, 2)], E[(b, 3)])
                            pr[(b, 1)] = p[:]
                        elif r == 5:
                            p = ppool.tile([128, 2048], bf, tag="pp")
                            nc.gpsimd.tensor_add(p[:], E[(b, 4)], E[(b, 5)])
                            pr[(b, 2)] = p[:]
                        elif r == 7:
                            p = ppool.tile([128, 2048], bf, tag="pp")
                            nc.gpsimd.tensor_add(p[:], E[(b, 6)], E[(b, 7)])
                            m1 = capool.tile([128, 2048], bf, tag="ca")
                            nc.vector.tensor_add(m1[:], pr[(b, 0)], pr[(b, 1)])
                            m2 = ppool.tile([128, 2048], bf, tag="pp")
                            nc.gpsimd.tensor_add(m2[:], pr[(b, 2)], p[:])
                            m3 = capool.tile([128, 2048], bf, tag="ca")
                            nc.vector.tensor_add(m3[:], m1[:], m2[:])
                            nc.sync.dma_start(colacc[b], m3[:])
                    else:
                        if r == 5:
                            p = ppool.tile([128, 2048], bf, tag="pp")
                            nc.gpsimd.tensor_add(p[:], E[(b, 4)], E[(b, 5)])
                            pr[(b, 0)] = p[:]
                        elif r == 7:
                            p = ppool.tile([128, 2048], bf, tag="pp")
                            nc.gpsimd.tensor_add(p[:], E[(b, 6)], E[(b, 7)])
                            m1 = capool.tile([128, 2048], bf, tag="ca")
                            nc.vector.tensor_add(m1[:], ch[b], pr[(b, 0)])
                            m2 = capool.tile([128, 2048], bf, tag="ca")
                            nc.vector.tensor_add(m2[:], m1[:], p[:])
                            nc.sync.dma_start(colacc[b], m2[:])

        nc.sync.dma_start(rowacc[:], rowacc_sb[:])
        nc.sync.dma_start(bandrow[:], bandrow_sb[:])

    if split_waits:
        _split_multi_waits(nc)
    return nc


def _prep_inputs(za8, zt8, pid_s):
    """Build the per-core input maps (fp8, per-core block permutation)."""
    zt8T = np.ascontiguousarray(zt8.T)  # (256, 8192) fp8
    ztT_all = np.ascontiguousarray(
        zt8T.reshape(2, 128, NB, CPB, 512).transpose(2, 1, 0, 3, 4)
    )  # (NB, 128, 2, CPB, 512)

    pidp = np.full(B + 2 * PAD, -1, dtype=np.int64)
    pidp[PAD:PAD + B] = pid_s
    zt8T_pad = np.zeros((D, B + 2 * PAD), dtype=FP8)
    zt8T_pad[:, PAD:PAD + B] = zt8T

    # SW-interleaved DoubleRow weight layout:
    #   zaT[p, r, 2*(127-m)+i] = za8[r0 + r*128 + m, i*128 + p]
    m_idx = np.arange(128)
    in_maps = []
    for c in range(NCORES):
        r0 = c * ROWS
        # za8 rows for this core, viewed [r, m, i, p]
        za_r = za8[r0:r0 + ROWS].reshape(RT, 128, 2, 128)
        # target [p, r, c] with c = 2*(127-m)+i
        zaTc = np.zeros((128, RT, 256), dtype=FP8)
        for i in range(2):
            # c = 2*(127-m)+i  for m = 0..127
            cols = 2 * (127 - m_idx) + i
            zaTc[:, :, cols] = za_r[:, :, i, :].transpose(2, 0, 1)
        zaTc = np.ascontiguousarray(zaTc)
        band = zt8T_pad[:, r0:r0 + Wb]
        ztTbc = np.ascontiguousarray(band.reshape(2, 128, Wb).transpose(1, 0, 2))
        mask = np.zeros((128, RT, W), dtype=BF16)
        for r in range(RT):
            rows = pid_s[r0 + r * 128: r0 + (r + 1) * 128]
            cols = pidp[r0 + r * 128: r0 + r * 128 + W]
            mask[:, r, :] = (rows[:, None] == cols[None, :]).astype(BF16)
        perm = [(c // 2 + lb) % NB for lb in range(NB)]
        ztTc = np.ascontiguousarray(ztT_all[perm])
        in_maps.append({"zaT": zaTc, "ztT": ztTc, "ztTb": ztTbc, "maskb": mask})
    return in_maps


def _numpy_fallback(z_a, z_t, patient_ids):
    z_a = np.asarray(z_a, np.float64)
    z_t = np.asarray(z_t, np.float64)
    pid = np.asarray(patient_ids)
    sim = (z_a @ z_t.T) / TEMPERATURE
    cross = pid[:, None] != pid[None, :]

    def direction(sim, cross):
        n = sim.shape[0]
        pos = np.diagonal(sim)
        mask = cross | np.eye(n, dtype=bool)
        neg = np.where(mask, sim, -np.inf)
        m = neg.max(axis=1)
        lse = np.log(np.exp(neg - m[:, None]).sum(axis=1)) + m
        row_loss = lse - pos
        valid = cross.any(axis=1)
        cnt = valid.sum()
        return (row_loss[valid].sum() / cnt) if cnt > 0 else 0.0

    loss = 0.5 * (direction(sim, cross) + direction(sim.T, cross.T))
    return np.asarray(loss, dtype=np.float32)


def kernel(z_a, z_t, patient_ids):
    from concourse.bass_utils import run_bass_kernel_spmd

    z_a = np.asarray(z_a)
    z_t = np.asarray(z_t)
    pid = np.asarray(patient_ids)
    assert z_a.shape == (B, D) and z_t.shape == (B, D)

    # Sort rows by patient id so same-patient pairs live in a diagonal band.
    perm = np.argsort(pid, kind="stable")
    pid_s = pid[perm].astype(np.int64)
    za_s = z_a[perm]
    zt_s = z_t[perm]

    _, counts = np.unique(pid_s, return_counts=True)
    gmax = int(counts.max())
    if gmax > 64:
        return _numpy_fallback(z_a, z_t, patient_ids)

    za8 = (za_s * 8.0).astype(FP8)
    zt8 = (zt_s * 8.0).astype(FP8)

    if "prog" not in _CACHE:
        _CACHE["prog"] = _build_program()
    nc = _CACHE["prog"]

    in_maps = _prep_inputs(za8, zt8, pid_s)
    r = run_bass_kernel_spmd(nc, in_maps, list(range(NCORES)))
    global _LAST_RESULT
    _LAST_RESULT = r
    res = r.results

    # ---- host-side assembly in float64
    pos = np.einsum(
        "ij,ij->i", za_s.astype(np.float64), zt_s.astype(np.float64)
    ) * SCALE
    pos_exp = np.exp(pos - C)

    rowS = np.zeros(B, dtype=np.float64)
    colS = np.zeros(B, dtype=np.float64)
    B_col = np.zeros(B, dtype=np.float64)
    for c in range(NCORES):
        ra = res[c]["rowacc"].astype(np.float64)  # [128, NB*RT]
        rs = np.zeros((RT, 128), dtype=np.float64)
        for lb in range(NB):
            k = K_BLK[lb]
            sl = ra[:, lb * RT: (lb + 1) * RT]
            rs[0] += sl[:, 0]
            for rr in range(1, RT):
                if rr <= k:  # chain accums: difference consecutive entries
                    rs[rr] += sl[:, rr] - sl[:, rr - 1]
                else:        # direct per-tile row sums
                    rs[rr] += sl[:, rr]
        rowS[c * ROWS:(c + 1) * ROWS] = rs.reshape(-1)

        cacc = res[c]["colacc"].astype(np.float64)  # [NB, 128, 2048]
        for lb in range(NB):
            g = (c // 2 + lb) % NB
            colS[g * 2048:(g + 1) * 2048] += cacc[lb].sum(axis=0)

        bc = res[c]["bandstk"].reshape(128, RT, W).astype(np.float64).sum(axis=0)
        for rr in range(RT):
            g0 = c * ROWS + rr * 128 - PAD
            lo = max(0, -g0)
            hi = min(W, B - g0)
            B_col[g0 + lo:g0 + hi] += bc[rr, lo:hi]

    B_row = np.concatenate(
        [res[c]["bandrow"].T.reshape(-1) for c in range(NCORES)]
    ).astype(np.float64)

    Sa = np.maximum(rowS - B_row + pos_exp, 1e-300)
    St = np.maximum(colS - B_col + pos_exp, 1e-300)
    row_loss_a = C + np.log(Sa) - pos
    row_loss_t = C + np.log(St) - pos

    uniq, inv, cnts = np.unique(pid_s, return_inverse=True, return_counts=True)
    group_sizes = cnts[inv]
    valid = group_sizes < B
    cnt = int(valid.sum())
    if cnt > 0:
        loss_a = row_loss_a[valid].sum() / cnt
        loss_t = row_loss_t[valid].sum() / cnt
    else:
        loss_a = loss_t = 0.0

    return np.asarray((loss_a + loss_t) / 2.0, dtype=np.float32)


# revision 35
# speedup vs baseline: 1.0948x; 1.0948x over previous
"""Cross-modal contrastive loss on 8 Trainium2 NeuronCores.

Strategy (B=8192, d=256 hardcoded):
  * Host sorts rows by patient id (same-patient mask becomes a narrow band),
    scales projections by 8 and quantizes to fp8-e4m3.
  * Each core owns a 1024-row slice of z_a and the full z_t. Main matmuls run
    in fp8 DoubleRow mode (full 256-contraction in one pass, 2x PE rate).
  * exp(sim/T - C) with a fixed offset C (|sim|<=1) is computed per
    (128, 2048) tile either on ACT (table exp -> bf16) or on DVE via the
    Schraudolph bit trick (i16 = A*psum + B, reinterpreted as bf16).
  * Column sums: per block, a serial DVE fold chain accumulates the 8 row
    tiles into a bf16 colacc; a ones-matmul partition-reduces it to [1,2048]
    psum which is DMA'd straight to HBM.
  * Row sums: every fold carries accum_out (sum over the free dim of the
    fold OUTPUT = cumulative block-row-sums); the host recovers per-tile row
    sums by differencing consecutive fold accums. Row-tile 0 of each block
    uses the ACT activation's accum_out directly.
  * The same-patient band is recomputed over a small window with identical
    fp8/ACT arithmetic and subtracted on the host. Per-core column-block
    permutation puts the band columns always in local blocks 0/1, so one
    SPMD program serves all cores (Schraudolph tiles live in blocks 2/3,
    never overlapping band columns).
"""

import numpy as np
import ml_dtypes

TEMPERATURE = 0.03
SCALE = 1.0 / TEMPERATURE
C = SCALE + 0.01  # fixed logsumexp offset; logits are <= SCALE * (1 + eps)
B = 8192
D = 256
NCORES = 8
ROWS = B // NCORES          # 1024 rows per core
RT = ROWS // 128            # 8 row-tiles per core
NB = 4                      # column blocks of 2048
CPB = 4                     # 512-chunks per block
PAD, W = 64, 256
Wb = ROWS + 2 * PAD
BF16 = ml_dtypes.bfloat16
FP8 = ml_dtypes.float8_e4m3

PSC = SCALE / 64.0          # psum holds 64*sim (inputs pre-scaled by 8)
# Schraudolph constants: i16 = trunc(S1 * psum + S2) viewed as bf16
# approximates exp(PSC*psum - C).  S1 = (128/ln2)*PSC,
# S2 = 16256 + 0.5(trunc->round) - 128*sigma - (128/ln2)*C
SCH_A = 128.0 / np.log(2.0)
SCH_SIGMA = 0.0575
S1 = SCH_A * PSC
S2 = 16256.5 - 128.0 * SCH_SIGMA - SCH_A * C
# (block, row-tile) pairs exp'd on DVE via Schraudolph. Band columns live in
# local block 0 (all cores) and (r=7, block 1) (odd cores), which must use
# the ACT path (bitwise match with the band phase); everything else is fair
# game. Spread across the timeline to balance ACT/DVE.
# Per-block Schraudolph chain-prefix length: tiles r=1..k of each block are
# exp'd on DVE; their folds are STT-with-accum whose cumulative row sums are
# differenced on the host. Block 0 holds the band columns -> ACT only.
K_BLK = [0, 3, 3, 3]
SCH_TILES = {(b, r) for b in range(4) for r in range(1, K_BLK[b] + 1)}

_LDW_PATCHED = False


def _install_ldw_opt_patch():
    """walrus ships with --enable-ldw-opt=false; with 8 consecutive
    same-weight matmuls per row-tile the dedupe is a big PE win."""
    global _LDW_PATCHED
    if _LDW_PATCHED:
        return
    import concourse.bass_utils as bu

    orig = bu.run_command

    def patched(cmd, **kw):
        if isinstance(cmd, list):
            cmd = ["--enable-ldw-opt=true" if c == "--enable-ldw-opt=false"
                   else c for c in cmd]
        return orig(cmd, **kw)

    bu.run_command = patched
    _LDW_PATCHED = True

_CACHE = {}


def _install_drain_patch():
    """walrus accepts at most one sync-wait per CTRL instruction, but
    TileContext's exit drain collects one wait per outstanding semaphore.
    Spread the waits across nop instructions, one wait each."""
    import bass_rust
    import concourse.tile as tile_mod
    from concourse.vector_clock import ScopedClock

    if getattr(tile_mod.TileContext, "_drain_patch_installed", False):
        return

    def _patched(self, tick_clock, wait_clock):
        nc = self.nc
        probe = nc.sync.nop(nofuse=True)
        wait_clock.add_sem_waits(
            probe.ins, ScopedClock({None: tick_clock.global_clock})
        )
        si = probe.ins.sync_info
        waits = list(si.on_wait) if si is not None else []
        if len(waits) > 1:
            si.on_wait = waits[:1]
            for w in waits[1:]:
                extra = nc.sync.nop(nofuse=True)
                extra.ins.sync_info = bass_rust.SyncInfo(on_wait=[w], on_update=[])
        nc.sync.drain()
        nc.all_engine_barrier()
        popped = nc._tile_sem_poison_stack.pop()
        assert popped is self._sem_poison
        nc.clear_and_free_semaphores(list(self.sems.allocated().values()))
        nc.all_engine_barrier()

    tile_mod.TileContext._drain_and_barrier = _patched
    tile_mod.TileContext._drain_patch_installed = True


def _split_multi_waits(nc):
    """walrus in this container accepts at most one sync-wait per instruction.
    Hoist extra waits onto same-engine nops inserted just before the
    instruction (engine streams are in-order, so the waits still gate it)."""
    import bass_rust

    n = 0
    for fn in nc.m.functions:
        for bb in fn.blocks:
            insts = list(bb.instructions)
            out = []
            for inst in insts:
                si = inst.sync_info
                if si is not None and len(si.on_wait) > 1:
                    waits = list(si.on_wait)
                    for w in waits[:-1]:
                        n += 1
                        nop = bass_rust.InstNoOp(
                            name=f"I-waitsplit-{n}", ins=[], outs=[]
                        )
                        nop.engine = inst.engine
                        nop.sync_info = bass_rust.SyncInfo(
                            on_wait=[w], on_update=[]
                        )
                        out.append(nop)
                    si.on_wait = waits[-1:]
                out.append(inst)
            if n:
                bb.instructions = out
    return n


def _build_program(split_waits=True):
    from contextlib import ExitStack
    import concourse.bass as bass
    import concourse.tile as tile
    from concourse import mybir

    _install_drain_patch()

    nc = bass.Bass()
    bf = mybir.dt.bfloat16
    f32 = mybir.dt.float32
    fp8 = mybir.dt.float8e4
    i16 = mybir.dt.int16
    DR = mybir.MatmulPerfMode.DoubleRow

    # Drop preamble memsets for const APs this program never uses.
    drop = ("const-float32-1.0", "const-bfloat16-1.0", "const-uint8-127")
    bb0 = nc.m.functions[0].blocks[0]
    bb0.instructions = [
        i for i in bb0.instructions
        if not (i.opcode == "Memset"
                and any(d in str(i.outs[0]) for d in drop))
    ]

    zaT = nc.declare_dram_parameter("zaT", [128, 2, ROWS], fp8, isOutput=False)
    ztT = nc.declare_dram_parameter("ztT", [NB, 128, 2, CPB, 512], fp8, isOutput=False)
    ztTb = nc.declare_dram_parameter("ztTb", [128, 2, Wb], fp8, isOutput=False)
    maskb = nc.declare_dram_parameter("maskb", [128, RT, W], bf, isOutput=False)

    rowacc = nc.declare_dram_parameter("rowacc", [128, NB * RT], f32, isOutput=True)
    bandrow = nc.declare_dram_parameter("bandrow", [128, RT], f32, isOutput=True)
    colacc = nc.declare_dram_parameter("colacc", [NB, 128, 2048], bf, isOutput=True)
    bandstk = nc.declare_dram_parameter("bandstk", [128, RT * W], bf, isOutput=True)

    with ExitStack() as ctx:
        tc = ctx.enter_context(tile.TileContext(nc))
        singles = ctx.enter_context(tc.tile_pool(name="singles", bufs=1))
        ztpool = ctx.enter_context(tc.tile_pool(name="ztpool", bufs=4))
        exppool = ctx.enter_context(tc.tile_pool(name="exppool", bufs=13))
        spool = ctx.enter_context(tc.tile_pool(name="spool", bufs=6))
        capool = ctx.enter_context(tc.tile_pool(name="capool", bufs=7))
        ppool = ctx.enter_context(tc.tile_pool(name="ppool", bufs=5))
        bpool = ctx.enter_context(tc.tile_pool(name="bpool", bufs=2))

        biasC = singles.tile([128, 1], f32)
        nc.vector.memset(biasC[:], -C)
        # Dummy activation: pull the exp table load off the critical path.
        warm = singles.tile([128, 1], f32)
        nc.scalar.activation(warm[:], biasC[:], mybir.ActivationFunctionType.Exp)

        rowacc_sb = singles.tile([128, NB * RT], f32)
        bandrow_sb = singles.tile([128, RT], f32)
        bandstack = singles.tile([128, RT, W], bf)

        # DMA order: band phase (first compute) needs zaT + the band windows.
        zaT0 = singles.tile([128, 2, 256], fp8)
        nc.sync.dma_start(zaT0[:], zaT[:, :, 0:256])
        ztTb_sb = singles.tile([128, 2, Wb], fp8)
        nc.sync.dma_start(ztTb_sb[:], ztTb[:])
        zaT1 = singles.tile([128, 2, 768], fp8)
        nc.sync.dma_start(zaT1[:], zaT[:, :, 256:ROWS])
        maskb_sb = singles.tile([128, RT, W], bf)
        nc.gpsimd.dma_start(maskb_sb[:], maskb[:])

        def za_dr(r):
            # [128, 2, 128] DoubleRow stationary slice for row-tile r
            if r < 2:
                return zaT0[:, :, r * 128:(r + 1) * 128]
            return zaT1[:, :, (r - 2) * 128:(r - 1) * 128]

        pmain = ctx.enter_context(tc.tile_pool(name="pmain", bufs=2, space="PSUM"))

        # ---- band phase: same-patient window, identical fp8/ACT arithmetic
        for r0 in range(0, RT, 2):
            pb = pmain.tile([128, 2048], f32, tag="pm")
            for i in range(2):
                w0 = 256 * (r0 // 2) + i * 128
                nc.tensor.matmul(
                    pb[:, i * W:(i + 1) * W],
                    za_dr(r0 + i),
                    ztTb_sb[:, :, w0:w0 + W],
                    start=True, stop=True,
                    perf_mode=DR,
                    skip_group_check=True,
                )
            exp_b = bpool.tile([128, 2 * W], bf, tag="exp_b")
            nc.scalar.activation(
                exp_b[:], pb[:, :2 * W], mybir.ActivationFunctionType.Exp,
                bias=biasC[:], scale=PSC,
            )
            for i in range(2):
                nc.vector.scalar_tensor_tensor(
                    out=bandstack[:, r0 + i, :],
                    in0=exp_b[:, i * W:(i + 1) * W],
                    scalar=1.0,
                    in1=maskb_sb[:, r0 + i, :],
                    op0=mybir.AluOpType.mult,
                    op1=mybir.AluOpType.mult,
                    accum_out=bandrow_sb[:, r0 + i:r0 + i + 1],
                )

        # band column partial sums: ship bandstack to HBM, reduce on host
        nc.sync.dma_start(
            bandstk[:], bandstack[:].rearrange("p r w -> p (r w)")
        )

        # ---- main phase: 4 column blocks of 2048, processed as two pairs
        # (r-major within a pair so the PE reuses each row-tile's weights
        # for 8 consecutive matmuls -> LDWEIGHTS fully hidden).
        ztbs = []
        for b in range(NB):
            ztb = ztpool.tile([128, 2, CPB, 512], fp8, tag="ztb")
            nc.sync.dma_start(ztb[:], ztT[b])
            ztbs.append(ztb)
        E = {}       # (b, r) -> AP of exp tile (bf16 view)
        # Delayed fold schedule: block b's folds are emitted interleaved
        # with block b+1's tile stream, so every fold input is old by the
        # time the op reaches its engine queue head (no convoy stalls).
        # Per block (k = sch prefix len): sch folds r=1..k are DVE STT with
        # accum (host diffs); pairs go to Pool; merges to DVE.
        def fold_ops(b):
            k = K_BLK[b]
            ops = []
            if k == 0:
                def p01():
                    t = capool.tile([128, 2048], bf, tag="ca")
                    nc.vector.tensor_add(t[:], E[(b, 0)], E[(b, 1)])
                    return ("p01", t[:])
                def p23():
                    t = ppool.tile([128, 2048], bf, tag="pp")
                    nc.gpsimd.tensor_add(t[:], E[(b, 2)], E[(b, 3)])
                    return ("p23", t[:])
                def p45():
                    t = ppool.tile([128, 2048], bf, tag="pp")
                    nc.gpsimd.tensor_add(t[:], E[(b, 4)], E[(b, 5)])
                    return ("p45", t[:])
                def p67():
                    t = ppool.tile([128, 2048], bf, tag="pp")
                    nc.gpsimd.tensor_add(t[:], E[(b, 6)], E[(b, 7)])
                    return ("p67", t[:])
                def q1(st):
                    t = capool.tile([128, 2048], bf, tag="ca")
                    nc.vector.tensor_add(t[:], st["p01"], st["p23"])
                    return ("q1", t[:])
                def q2(st):
                    t = capool.tile([128, 2048], bf, tag="ca")
                    nc.vector.tensor_add(t[:], st["p45"], st["p67"])
                    return ("q2", t[:])
                def fin(st):
                    t = capool.tile([128, 2048], bf, tag="ca")
                    nc.vector.tensor_add(t[:], st["q1"], st["q2"])
                    nc.sync.dma_start(colacc[b], t[:])
                    return ("fin", t[:])
                ops = [p01, p23, p45, p67, q1, q2, fin]
            else:
                def mk_stt(r):
                    def stt(st):
                        prev = st["ch"] if r > 1 else E[(b, 0)]
                        t = capool.tile([128, 2048], bf, tag="ca")
                        nc.vector.scalar_tensor_tensor(
                            out=t[:],
                            in0=E[(b, r)],
                            scalar=1.0,
                            in1=prev,
                            op0=mybir.AluOpType.mult,
                            op1=mybir.AluOpType.add,
                            accum_out=rowacc_sb[:, b * RT + r:b * RT + r + 1],
                        )
                        return ("ch", t[:])
                    return stt
                def p45(st):
                    t = ppool.tile([128, 2048], bf, tag="pp")
                    nc.gpsimd.tensor_add(t[:], E[(b, k + 1)], E[(b, k + 2)])
                    return ("p45", t[:])
                def p67(st):
                    t = ppool.tile([128, 2048], bf, tag="pp")
                    nc.gpsimd.tensor_add(t[:], E[(b, k + 3)], E[(b, k + 4)])
                    return ("p67", t[:])
                def m1(st):
                    t = capool.tile([128, 2048], bf, tag="ca")
                    nc.vector.tensor_add(t[:], st["ch"], st["p45"])
                    return ("m1", t[:])
                def m2(st):
                    t = capool.tile([128, 2048], bf, tag="ca")
                    nc.vector.tensor_add(t[:], st["m1"], st["p67"])
                    nc.sync.dma_start(colacc[b], t[:])
                    return ("m2", t[:])
                ops = [mk_stt(r) for r in range(1, k + 1)] + [p45, p67, m1, m2]
            return ops

        import inspect as _inspect

        fold_state = {}
        pending = []  # list of (b, op) to drain, one per tile slot

        def run_fold(bop):
            b, op = bop
            st = fold_state.setdefault(b, {})
            if len(_inspect.signature(op).parameters) == 0:
                key, ap = op()
            else:
                key, ap = op(st)
            st[key] = ap

        for b in range(NB):
            ztb = ztbs[b]
            k = K_BLK[b]
            for r in range(RT):
                pm = pmain.tile([128, 2048], f32, tag="pm")
                for jj in range(CPB):
                    nc.tensor.matmul(
                        pm[:, jj * 512:(jj + 1) * 512],
                        za_dr(r),
                        ztb[:, :, jj, :],
                        start=True, stop=True,
                        perf_mode=DR,
                        skip_group_check=True,
                    )
                racc = rowacc_sb[:, b * RT + r:b * RT + r + 1]
                if (b, r) in SCH_TILES:
                    st = spool.tile([128, 2048], i16, tag="sch")
                    nc.vector.tensor_scalar(
                        st[:], pm[:], S1, S2,
                        mybir.AluOpType.mult, mybir.AluOpType.add,
                    )
                    E[(b, r)] = st[:].bitcast(bf)
                else:
                    ex = exppool.tile([128, 2048], bf, tag="exp")
                    nc.scalar.activation(
                        ex[:], pm[:], mybir.ActivationFunctionType.Exp,
                        bias=biasC[:], scale=PSC,
                        accum_out=racc,
                    )
                    E[(b, r)] = ex[:]
                # drain one delayed fold op of the previous block per tile
                if pending:
                    run_fold(pending.pop(0))
                # last block folds promptly (no next stream to hide behind)
                if b == NB - 1:
                    if 1 <= r <= k:
                        run_fold((b, fold_ops(b)[r - 1]))
                    elif r == k + 2:
                        run_fold((b, fold_ops(b)[k]))      # p45
                    elif r == k + 4:
                        for op in fold_ops(b)[k + 1:]:     # p67, m1, m2
                            run_fold((b, op))
            if b < NB - 1:
                pending.extend((b, op) for op in fold_ops(b))
        while pending:
            run_fold(pending.pop(0))

        nc.sync.dma_start(rowacc[:], rowacc_sb[:])
        nc.sync.dma_start(bandrow[:], bandrow_sb[:])

    if split_waits:
        _split_multi_waits(nc)
    return nc


def _prep_inputs(za8, zt8, pid_s):
    """Build the per-core input maps (fp8, per-core block permutation)."""
    zt8T = np.ascontiguousarray(zt8.T)  # (256, 8192) fp8
    ztT_all = np.ascontiguousarray(
        zt8T.reshape(2, 128, NB, CPB, 512).transpose(2, 1, 0, 3, 4)
    )  # (NB, 128, 2, CPB, 512)

    pidp = np.full(B + 2 * PAD, -1, dtype=np.int64)
    pidp[PAD:PAD + B] = pid_s
    zt8T_pad = np.zeros((D, B + 2 * PAD), dtype=FP8)
    zt8T_pad[:, PAD:PAD + B] = zt8T

    in_maps = []
    for c in range(NCORES):
        r0 = c * ROWS
        zaTc = np.ascontiguousarray(
            za8[r0:r0 + ROWS].T.reshape(2, 128, ROWS).transpose(1, 0, 2)
        )  # (128, 2, ROWS)
        band = zt8T_pad[:, r0:r0 + Wb]
        ztTbc = np.ascontiguousarray(band.reshape(2, 128, Wb).transpose(1, 0, 2))
        mask = np.zeros((128, RT, W), dtype=BF16)
        for r in range(RT):
            rows = pid_s[r0 + r * 128: r0 + (r + 1) * 128]
            cols = pidp[r0 + r * 128: r0 + r * 128 + W]
            mask[:, r, :] = (rows[:, None] == cols[None, :]).astype(BF16)
        perm = [(c // 2 + lb) % NB for lb in range(NB)]
        ztTc = np.ascontiguousarray(ztT_all[perm])
        in_maps.append({"zaT": zaTc, "ztT": ztTc, "ztTb": ztTbc, "maskb": mask})
    return in_maps


def _numpy_fallback(z_a, z_t, patient_ids):
    z_a = np.asarray(z_a, np.float64)
    z_t = np.asarray(z_t, np.float64)
    pid = np.asarray(patient_ids)
    sim = (z_a @ z_t.T) / TEMPERATURE
    cross = pid[:, None] != pid[None, :]

    def direction(sim, cross):
        n = sim.shape[0]
        pos = np.diagonal(sim)
        mask = cross | np.eye(n, dtype=bool)
        neg = np.where(mask, sim, -np.inf)
        m = neg.max(axis=1)
        lse = np.log(np.exp(neg - m[:, None]).sum(axis=1)) + m
        row_loss = lse - pos
        valid = cross.any(axis=1)
        cnt = valid.sum()
        return (row_loss[valid].sum() / cnt) if cnt > 0 else 0.0

    loss = 0.5 * (direction(sim, cross) + direction(sim.T, cross.T))
    return np.asarray(loss, dtype=np.float32)


def kernel(z_a, z_t, patient_ids):
    from concourse.bass_utils import run_bass_kernel_spmd

    z_a = np.asarray(z_a)
    z_t = np.asarray(z_t)
    pid = np.asarray(patient_ids)
    assert z_a.shape == (B, D) and z_t.shape == (B, D)

    # Sort rows by patient id so same-patient pairs live in a diagonal band.
    perm = np.argsort(pid, kind="stable")
    pid_s = pid[perm].astype(np.int64)
    za_s = z_a[perm]
    zt_s = z_t[perm]

    _, counts = np.unique(pid_s, return_counts=True)
    gmax = int(counts.max())
    if gmax > 64:
        return _numpy_fallback(z_a, z_t, patient_ids)

    za8 = (za_s * 8.0).astype(FP8)
    zt8 = (zt_s * 8.0).astype(FP8)

    if "prog" not in _CACHE:
        _CACHE["prog"] = _build_program()
    nc = _CACHE["prog"]

    in_maps = _prep_inputs(za8, zt8, pid_s)
    r = run_bass_kernel_spmd(nc, in_maps, list(range(NCORES)))
    global _LAST_RESULT
    _LAST_RESULT = r
    res = r.results

    # ---- host-side assembly in float64
    pos = np.einsum(
        "ij,ij->i", za_s.astype(np.float64), zt_s.astype(np.float64)
    ) * SCALE
    pos_exp = np.exp(pos - C)

    rowS = np.zeros(B, dtype=np.float64)
    colS = np.zeros(B, dtype=np.float64)
    B_col = np.zeros(B, dtype=np.float64)
    for c in range(NCORES):
        ra = res[c]["rowacc"].astype(np.float64)  # [128, NB*RT]
        rs = np.zeros((RT, 128), dtype=np.float64)
        for lb in range(NB):
            k = K_BLK[lb]
            sl = ra[:, lb * RT: (lb + 1) * RT]
            rs[0] += sl[:, 0]
            for rr in range(1, RT):
                if rr <= k:  # chain accums: difference consecutive entries
                    rs[rr] += sl[:, rr] - sl[:, rr - 1]
                else:        # direct per-tile row sums
                    rs[rr] += sl[:, rr]
        rowS[c * ROWS:(c + 1) * ROWS] = rs.reshape(-1)

        cacc = res[c]["colacc"].astype(np.float64)  # [NB, 128, 2048]
        for lb in range(NB):
            g = (c // 2 + lb) % NB
            colS[g * 2048:(g + 1) * 2048] += cacc[lb].sum(axis=0)

        bc = res[c]["bandstk"].reshape(128, RT, W).astype(np.float64).sum(axis=0)
        for rr in range(RT):
            g0 = c * ROWS + rr * 128 - PAD
            lo = max(0, -g0)
            hi = min(W, B - g0)
            B_col[g0 + lo:g0 + hi] += bc[rr, lo:hi]

    B_row = np.concatenate(
        [res[c]["bandrow"].T.reshape(-1) for c in range(NCORES)]
    ).astype(np.float64)

    Sa = np.maximum(rowS - B_row + pos_exp, 1e-300)
    St = np.maximum(colS - B_col + pos_exp, 1e-300)
    row_loss_a = C + np.log(Sa) - pos
    row_loss_t = C + np.log(St) - pos

    uniq, inv, cnts = np.unique(pid_s, return_inverse=True, return_counts=True)
    group_sizes = cnts[inv]
    valid = group_sizes < B
    cnt = int(valid.sum())
    if cnt > 0:
        loss_a = row_loss_a[valid].sum() / cnt
        loss_t = row_loss_t[valid].sum() / cnt
    else:
        loss_a = loss_t = 0.0

    return np.asarray((loss_a + loss_t) / 2.0, dtype=np.float32)
